# revision 9
# baseline (speedup 1.0000x reference)
"""Trainium2 Bass kernel for nn_DKEncoder (CokeBert-style 2-layer DK encoder).

Math per batch element b (see reference):
  q0 = q[b,0]                                    [768]
  qi_l = tanh(Wq_l @ q0 + bq_l)                  [100]   l in {2,1}
  w_l  = (Wk_l.T @ qi_l) / 10                    [100]   (Wk folded into a matvec)
  L0:  s2[e,n1,n2] = k2[e,n1,n2,:] . w2
       attn2 = masked leaky-softmax over n2
       c2[e,n1,:] = sum_n2 attn2 * v2[e,n1,n2,:]          [E,16,100]
  L1:  s1 = k1 . w1; attn1 likewise over n1
       c1[e,:] = sum_n1 attn1 * [v1|c2][e,n1,:]           [E,200]
  scatter: out[s] = c1[rank[s]] if input_ent[s]!=0 else 0

Sharding: data-parallel over B=8, one batch element per NeuronCore.

Layout: "e-per-partition" tiling. Every heavy tile is [128 partitions = e,
16 free-rows = neighbor, 100 free-cols = feature], DMA'd with one contiguous
6400B run per partition (descriptor-efficient). Softmax groups live along the
free dim (per-partition bias/accum on ACT), the weighted neighbor-sum is a
strided segmented reduce on DVE, and combined2 lands directly in the layout
layer 1 consumes. Big DMAs alternate between the SP and ACT HWDGE rings.
"""
import numpy as np

import concourse.bass as bass
import concourse.mybir as mybir
import concourse.tile as tile

F32 = mybir.dt.float32
I32 = mybir.dt.int32
AF = mybir.ActivationFunctionType
OP = mybir.AluOpType
AX = mybir.AxisListType

P = 128
D = 100            # K_V_DIM
NB = 16            # neighbors per group (N1 = N2 = 16)
S = 256            # sequence length
Q = 768            # query dim
INV_SQRT_D = 0.1   # 1/sqrt(100)


def build_nc(E=256, k2_mul="dve", v2_wt="dve", split_waits=True, repeat=1):
    """Build the single-core Bass program.

    k2_mul: 'dve' | 'gpsimd' | 'dma'(multiply during DMA via SWDGE CCE)
    v2_wt:  'dve' | 'gpsimd'  — engine for the v2 attn-weighting multiply
    """
    R0 = E * NB * NB           # layer-0 rows (65536)
    R1 = E * NB                # layer-1 rows (4096)
    ET = E // P                # e-tiles (2)
    NT0 = ET * NB              # layer-0 tiles (T, n1) (32)
    NT1 = ET                   # layer-1 tiles (2)
    assert E % P == 0

    nc = bass.Bass()

    # ---- I/O ----
    ent = nc.dram_tensor("ent", [1, S], I32, kind="ExternalInput")
    q0 = nc.dram_tensor("q0", [1, Q], F32, kind="ExternalInput")
    k1 = nc.dram_tensor("k1", [R1, D], F32, kind="ExternalInput")
    v1 = nc.dram_tensor("v1", [R1, D], F32, kind="ExternalInput")
    k2 = nc.dram_tensor("k2", [R0, D], F32, kind="ExternalInput")
    v2 = nc.dram_tensor("v2", [R0, D], F32, kind="ExternalInput")
    Wq2T = nc.dram_tensor("Wq2T", [Q, D], F32, kind="ExternalInput")
    bq2 = nc.dram_tensor("bq2", [D], F32, kind="ExternalInput")
    Wk2 = nc.dram_tensor("Wk2", [D, D], F32, kind="ExternalInput")
    Wq1T = nc.dram_tensor("Wq1T", [Q, D], F32, kind="ExternalInput")
    bq1 = nc.dram_tensor("bq1", [D], F32, kind="ExternalInput")
    Wk1 = nc.dram_tensor("Wk1", [D, D], F32, kind="ExternalInput")
    identity = nc.dram_tensor("identity", [P, P], F32, kind="ExternalInput")
    iota_e = nc.dram_tensor("iota_e", [P, ET], F32, kind="ExternalInput")
    outp = nc.dram_tensor("outp", [S, 2 * D], F32, kind="ExternalOutput")

    # 4-D row views: r = e*256 + n1*16 + n2  /  r1 = e*16 + n1
    k2v = k2[:].rearrange("(e n1 n2) c -> e n1 n2 c", n1=NB, n2=NB)
    v2v = v2[:].rearrange("(e n1 n2) c -> e n1 n2 c", n1=NB, n2=NB)
    k1v = k1[:].rearrange("(e n1) c -> e n1 c", n1=NB)
    v1v = v1[:].rearrange("(e n1) c -> e n1 c", n1=NB)

    with tile.TileContext(nc) as tc:
        with tc.tile_pool(name="cpool", bufs=1) as cp, \
             tc.tile_pool(name="stream", bufs=4) as st, \
             tc.tile_pool(name="work", bufs=2) as wk, \
             tc.tile_pool(name="psT", bufs=1, space="PSUM") as psT, \
             tc.tile_pool(name="psM", bufs=1, space="PSUM") as psM:

            def emit():
                # ---------- constants (ACT-funnelled: see _split_multi_waits) ----------
                ident0 = cp.tile([P, P], F32, tag="ident0")
                nc.sync.dma_start(out=ident0[:], in_=identity[:])
                ident = cp.tile([P, P], F32, tag="ident")
                nc.scalar.copy(out=ident[:], in_=ident0[:])
                iot = cp.tile([P, ET], F32, tag="iot")
                nc.sync.dma_start(out=iot[:], in_=iota_e[:])
                iotc = cp.tile([P, ET], F32, tag="iotc")
                nc.scalar.copy(out=iotc[:], in_=iot[:])
                ones = cp.tile([1, P], F32, tag="ones")
                nc.scalar.activation(out=ones[:], in_=ident[:1, :], func=AF.Identity,
                                     bias=1.0, scale=0.0)

                # ---------- q_i / w / wrep for both layers ----------
                q0c0 = cp.tile([P, 6], F32, tag="q0c0")
                nc.sync.dma_start(out=q0c0[:], in_=q0[:].rearrange("a (j p) -> (a p) j", p=P))
                q0c = cp.tile([P, 6], F32, tag="q0c")
                nc.scalar.copy(out=q0c[:], in_=q0c0[:])

                wreps = {}
                for lname, WqT_d, bq_d, Wk_d in (("2", Wq2T, bq2, Wk2), ("1", Wq1T, bq1, Wk1)):
                    wqt0 = cp.tile([P, 6, D], F32, tag=f"wqt0{lname}")
                    nc.sync.dma_start(out=wqt0[:], in_=WqT_d[:].rearrange("(j p) m -> p j m", p=P))
                    wqt = cp.tile([P, 6, D], F32, tag=f"wqt{lname}")
                    nc.scalar.copy(out=wqt[:], in_=wqt0[:])
                    bqc0 = cp.tile([D, 1], F32, tag=f"bqc0{lname}")
                    nc.sync.dma_start(out=bqc0[:], in_=bq_d[:].unsqueeze(1))
                    bqc = cp.tile([D, 1], F32, tag=f"bqc{lname}")
                    nc.scalar.copy(out=bqc[:], in_=bqc0[:])
                    wkt0 = cp.tile([D, D], F32, tag=f"wkt0{lname}")
                    nc.sync.dma_start(out=wkt0[:], in_=Wk_d[:])
                    wkt = cp.tile([D, D], F32, tag=f"wkt{lname}")
                    nc.scalar.copy(out=wkt[:], in_=wkt0[:])

                    qi_ps = psM.tile([D, 1], F32, tag="misc")
                    for j in range(6):
                        nc.tensor.matmul(out=qi_ps[:], lhsT=wqt[:, j, :], rhs=q0c[:, j:j + 1],
                                         start=(j == 0), stop=(j == 5))
                    qi = cp.tile([D, 1], F32, tag=f"qi{lname}")
                    nc.scalar.activation(out=qi[:], in_=qi_ps[:], func=AF.Tanh,
                                         bias=bqc[:], scale=1.0)

                    w_ps = psM.tile([1, D], F32, tag="misc")
                    nc.tensor.matmul(out=w_ps[:], lhsT=qi[:], rhs=wkt[:], start=True, stop=True)
                    wrow = cp.tile([1, D], F32, tag=f"wrow{lname}")
                    nc.scalar.activation(out=wrow[:], in_=w_ps[:], func=AF.Copy,
                                         scale=INV_SQRT_D)
                    wrr = cp.tile([1, NB * D], F32, tag=f"wrr{lname}")
                    nc.scalar.copy(out=wrr[:].rearrange("a (s c) -> a s c", c=D),
                                   in_=wrow[:].unsqueeze(1).to_broadcast([1, NB, D]))
                    wrep = cp.tile([P, NB * D], F32, tag=f"wrep{lname}")
                    for u in range(4):
                        wp_ps = psM.tile([P, 400], F32, tag="misc")
                        nc.tensor.matmul(out=wp_ps[:], lhsT=ones[:],
                                         rhs=wrr[:, 400 * u:400 * (u + 1)],
                                         start=True, stop=True)
                        nc.scalar.copy(out=wrep[:, 400 * u:400 * (u + 1)], in_=wp_ps[:])
                    wreps[lname] = wrep

                # ---------- scatter indices ----------
                ent_i = cp.tile([1, S], I32, tag="ent_i")
                nc.sync.dma_start(out=ent_i[:], in_=ent[:])
                ent_f = cp.tile([1, S], F32, tag="ent_f")
                nc.vector.tensor_copy(out=ent_f[:], in_=ent_i[:])
                mask = cp.tile([1, S], F32, tag="mask")
                nc.vector.tensor_scalar(out=mask[:], in0=ent_f[:], scalar1=0.0,
                                        scalar2=None, op0=OP.not_equal)
                csum = cp.tile([1, S], F32, tag="csum")
                nc.vector.tensor_tensor_scan(out=csum[:], data0=mask[:], data1=mask[:],
                                             initial=0.0, op0=OP.add, op1=OP.bypass)
                rank = cp.tile([1, S], F32, tag="rank")
                nc.vector.tensor_tensor(out=rank[:], in0=csum[:], in1=mask[:], op=OP.mult)
                nc.vector.tensor_scalar(out=rank[:], in0=rank[:], scalar1=-1.0,
                                        scalar2=float(E - 1), op0=OP.add, op1=OP.min)
                dmy = psT.tile([P, P], F32, tag="tp")
                nc.tensor.transpose(out=dmy[:P, :1], in_=rank[:, :P], identity=ident[:1, :1])
                rank_ps = psM.tile([P, S], F32, tag="misc")
                nc.tensor.matmul(out=rank_ps[:], lhsT=ones[:], rhs=rank[:], start=True, stop=True)
                PT = cp.tile([P, ET, S], F32, tag="PT")
                for kk in range(ET):
                    nc.vector.tensor_scalar(out=PT[:, kk, :], in0=rank_ps[:],
                                            scalar1=iotc[:, kk:kk + 1], scalar2=None,
                                            op0=OP.is_equal)

                # ---------- generic pieces ----------
                def scores_pass(tiles, wrep, Smat, is_l0):
                    """tiles: list of (dma_engine, src_ap). Scores -> Smat col 16*ti+j."""
                    for ti, (deng, src) in enumerate(tiles):
                        prod = wk.tile([P, NB * D], F32, tag="prod")
                        if is_l0 and k2_mul == "dma":
                            nc.scalar.copy(out=prod[:], in_=wrep[:])
                            nc.gpsimd.dma_start(out=prod[:].rearrange("p (s c) -> p s c", c=D),
                                                in_=src, accum_op=OP.mult)
                        else:
                            kt = st.tile([P, NB, D], F32, tag="kt")
                            deng.dma_start(out=kt[:], in_=src)
                            meng = nc.gpsimd if (is_l0 and k2_mul == "gpsimd") else nc.vector
                            meng.tensor_tensor(out=prod[:],
                                               in0=kt[:].rearrange("p s c -> p (s c)"),
                                               in1=wrep[:], op=OP.mult)
                        nc.vector.reduce_sum(out=Smat[:, NB * ti:NB * (ti + 1)],
                                             in_=prod[:].rearrange("p (s c) -> p s c", c=D),
                                             axis=AX.X)

                def softmax_block(Sm, Sp, ncols):
                    """Batched masked leaky-softmax over 16-groups along free dim."""
                    ng = ncols // NB
                    g3 = lambda t: t[:].rearrange("p (g j) -> p g j", j=NB)
                    zq = wk.tile([P, ncols], F32, tag="sx_zq")
                    nc.vector.tensor_scalar(out=zq[:], in0=Sm[:], scalar1=0.0,
                                            scalar2=-10000.0, op0=OP.is_equal, op1=OP.mult)
                    sm = wk.tile([P, ncols], F32, tag="sx_sm")
                    nc.vector.tensor_tensor(out=sm[:], in0=Sm[:], in1=zq[:], op=OP.add)
                    lt = wk.tile([P, ncols], F32, tag="sx_lt")
                    nc.vector.tensor_scalar(out=lt[:], in0=sm[:], scalar1=0.01,
                                            scalar2=None, op0=OP.mult)
                    lr = wk.tile([P, ncols], F32, tag="sx_lr")
                    nc.vector.tensor_tensor(out=lr[:], in0=sm[:], in1=lt[:], op=OP.max)
                    nm = wk.tile([P, ng], F32, tag="sx_nm")
                    nc.vector.tensor_reduce(out=nm[:], in_=g3(lr), axis=AX.X,
                                            op=OP.max, negate=True)
                    ex = wk.tile([P, ncols], F32, tag="sx_ex")
                    zz = wk.tile([P, ng], F32, tag="sx_zz")
                    for g in range(ng):
                        nc.scalar.activation(out=ex[:, NB * g:NB * (g + 1)],
                                             in_=lr[:, NB * g:NB * (g + 1)], func=AF.Exp,
                                             bias=nm[:, g:g + 1], scale=1.0,
                                             accum_out=zz[:, g:g + 1])
                    rz = wk.tile([P, ng], F32, tag="sx_rz")
                    nc.vector.reciprocal(out=rz[:], in_=zz[:])
                    at = wk.tile([P, ncols], F32, tag="sx_at")
                    nc.vector.tensor_tensor(out=g3(at), in0=g3(ex),
                                            in1=rz[:].unsqueeze(2).to_broadcast([P, ng, NB]),
                                            op=OP.mult)
                    mq = wk.tile([P, ncols], F32, tag="sx_mq")
                    nc.vector.tensor_scalar(out=mq[:], in0=at[:], scalar1=1.0 / NB,
                                            scalar2=None, op0=OP.not_equal)
                    nc.vector.tensor_tensor(out=Sp[:], in0=at[:], in1=mq[:], op=OP.mult)

                def weight_and_sum(vt_ap, attn_blk, out_blk, weng):
                    """wv = v * attn (bcast over c); out[p,c] = sum_j wv[p,j,c]."""
                    wv = wk.tile([P, NB, D], F32, tag="wv")
                    weng.tensor_tensor(out=wv[:], in0=vt_ap,
                                       in1=attn_blk.unsqueeze(2).to_broadcast([P, NB, D]),
                                       op=OP.mult)
                    # segmented reduce over j: view [p, c, j] (j strided by 100)
                    nc.vector.reduce_sum(
                        out=out_blk,
                        in_=wv[:].rearrange("p s c -> p c s"),
                        axis=AX.X)

                # ---------- layer 0 ----------
                l0_tiles = []
                for T in range(ET):
                    for n1 in range(NB):
                        deng = nc.sync if (T * NB + n1) % 2 == 0 else nc.scalar
                        l0_tiles.append((deng, k2v[P * T:P * (T + 1), n1, :, :]))
                S0 = cp.tile([P, NT0 * NB], F32, tag="S0")
                scores_pass(l0_tiles, wreps["2"], S0, True)
                S0p = cp.tile([P, NT0 * NB], F32, tag="S0p")
                softmax_block(S0, S0p, NT0 * NB)

                c2 = cp.tile([P, ET, NB, D], F32, tag="c2")
                v2eng = nc.gpsimd if v2_wt == "gpsimd" else nc.vector
                for T in range(ET):
                    for n1 in range(NB):
                        ti = T * NB + n1
                        deng = nc.scalar if ti % 2 == 0 else nc.sync
                        vt = st.tile([P, NB, D], F32, tag="vt")
                        deng.dma_start(out=vt[:], in_=v2v[P * T:P * (T + 1), n1, :, :])
                        weight_and_sum(vt[:], S0p[:, NB * ti:NB * (ti + 1)],
                                       c2[:, T, n1, :], v2eng)

                # ---------- layer 1 ----------
                l1_tiles = []
                for T in range(NT1):
                    deng = nc.sync if T % 2 == 0 else nc.scalar
                    l1_tiles.append((deng, k1v[P * T:P * (T + 1), :, :]))
                S1 = cp.tile([P, NT1 * NB], F32, tag="S1")
                scores_pass(l1_tiles, wreps["1"], S1, False)
                S1p = cp.tile([P, NT1 * NB], F32, tag="S1p")
                softmax_block(S1, S1p, NT1 * NB)

                c1 = cp.tile([P, ET, 2 * D], F32, tag="c1")
                for T in range(NT1):
                    deng = nc.scalar if T % 2 == 0 else nc.sync
                    vt1 = st.tile([P, NB, D], F32, tag="vt")
                    deng.dma_start(out=vt1[:], in_=v1v[P * T:P * (T + 1), :, :])
                    ab = S1p[:, NB * T:NB * (T + 1)]
                    weight_and_sum(vt1[:], ab, c1[:, T, 0:D], nc.vector)
                    weight_and_sum(c2[:, T, :, :], ab, c1[:, T, D:2 * D], nc.vector)

                # ---------- scatter ----------
                for hh in range(S // P):
                    ops = psM.tile([P, 2 * D], F32, tag="misc")
                    for kk in range(ET):
                        nc.tensor.matmul(out=ops[:], lhsT=PT[:, kk, P * hh:P * (hh + 1)],
                                         rhs=c1[:, kk, :], start=(kk == 0),
                                         stop=(kk == ET - 1))
                    osb = wk.tile([P, 2 * D], F32, tag="osb")
                    nc.scalar.copy(out=osb[:], in_=ops[:])
                    nc.sync.dma_start(out=outp[P * hh:P * (hh + 1), :], in_=osb[:])

            for _rep in range(repeat):
                emit()

    if split_waits:
        _split_multi_waits(nc)
    return nc


def _split_multi_waits(nc):
    """This walrus build allows at most ONE sync-wait command per
    instruction; hoist extras onto standalone EventSemaphore waits."""
    n = 0
    for bb in nc.m.functions[0].blocks:
        insts = bb.instructions
        i = 0
        while i < len(insts):
            ins = insts[i]
            si = ins.sync_info
            if si is not None and si.on_wait and len(si.on_wait) >= 2:
                extras, keep = list(si.on_wait[:-1]), [si.on_wait[-1]]
                for w in extras:
                    e = mybir.InstEventSemaphore(
                        name=nc.get_next_instruction_name(), ins=[], outs=[])
                    e.engine = ins.engine
                    e.sync_info = mybir.SyncInfo(on_wait=[w], on_update=[])
                    insts.insert(i, e)
                    i += 1
                    n += 1
                ins.sync_info = mybir.SyncInfo(on_wait=keep,
                                               on_update=list(si.on_update))
            i += 1
    return n


# ------------------------------------------------------------------
# host-side wrapper
# ------------------------------------------------------------------
_NC_CACHE = {}


def _get_nc(E=256, k2_mul="dve", v2_wt="dve", repeat=1):
    key = (E, k2_mul, v2_wt, repeat)
    if key not in _NC_CACHE:
        _NC_CACHE[key] = build_nc(E, k2_mul, v2_wt, repeat=repeat)
    return _NC_CACHE[key]


def _constants(E):
    ET = E // P
    ident = np.eye(P, dtype=np.float32)
    iot = np.zeros((P, ET), np.float32)
    for kk in range(ET):
        iot[:, kk] = np.arange(P) + P * kk
    return ident, iot


def _in_map_for_core(b, inputs, E):
    ident, iot = _constants(E)
    return {
        "ent": inputs["input_ent"][b].astype(np.int32).reshape(1, S),
        "q0": np.ascontiguousarray(inputs["q"][b, 0]).reshape(1, Q),
        "k1": np.ascontiguousarray(inputs["k1"][b]).reshape(E * NB, D),
        "v1": np.ascontiguousarray(inputs["v1"][b]).reshape(E * NB, D),
        "k2": np.ascontiguousarray(inputs["k2"][b]).reshape(E * NB * NB, D),
        "v2": np.ascontiguousarray(inputs["v2"][b]).reshape(E * NB * NB, D),
        "Wq2T": np.ascontiguousarray(inputs["Wq2"].T),
        "bq2": np.ascontiguousarray(inputs["bq2"]),
        "Wk2": np.ascontiguousarray(inputs["Wk2"]),
        "Wq1T": np.ascontiguousarray(inputs["Wq1"].T),
        "bq1": np.ascontiguousarray(inputs["bq1"]),
        "Wk1": np.ascontiguousarray(inputs["Wk1"]),
        "identity": ident,
        "iota_e": iot,
    }


def kernel(input_ent, q, k1, v1, k2, v2, Wq2, bq2, Wk2, Wq1, bq1, Wk1, **kw):
    from concourse.bass_utils import run_bass_kernel_spmd

    inputs = dict(input_ent=np.asarray(input_ent), q=np.asarray(q, np.float32),
                  k1=np.asarray(k1, np.float32), v1=np.asarray(v1, np.float32),
                  k2=np.asarray(k2, np.float32), v2=np.asarray(v2, np.float32),
                  Wq2=np.asarray(Wq2, np.float32), bq2=np.asarray(bq2, np.float32),
                  Wk2=np.asarray(Wk2, np.float32), Wq1=np.asarray(Wq1, np.float32),
                  bq1=np.asarray(bq1, np.float32), Wk1=np.asarray(Wk1, np.float32))
    B = inputs["input_ent"].shape[0]
    E = inputs["k1"].shape[1]
    nc = _get_nc(E, kw.get("k2_mul", "dve"), kw.get("v2_wt", "dve"))
    in_maps = [_in_map_for_core(b, inputs, E) for b in range(B)]
    res = run_bass_kernel_spmd(nc, in_maps, core_ids=list(range(B)),
                               trace=kw.get("trace", False))
    out = np.stack([res.results[b]["outp"] for b in range(B)], axis=0)
    if kw.get("return_res", False):
        return out, res
    return out
